# revision 2
# baseline (speedup 1.0000x reference)
"""Trainium2 Bass kernel for nn_DynamicAttentionModel.

Model math (see reference):
    z          = seed_emb[seeds]                          [B, Z]
    h          = relu(z @ hw1 + hb1)                      [B, H]
    coeffs_div = softmax(h @ hw2 + hb2, axis=1)           [B, NB]
    coeffs     = softmax(static_coeffs, 1) + coeffs_div   [B, NB]
    q/k/v      = einsum('bi,bj,ijk->bk', coeffs, feat, W{q,k,v})
    scores     = <q,k>/sqrt(D); attn = softmax over a SINGLE element == 1.0
    pooled     = attn * v == v
    logits     = pooled @ cw + cb

Because attn_map is softmax over one element it is identically 1.0, so q and k
never influence the output: logits depends only on the v-projection.

    logits[b] = sum_i coeffs[b,i] * (feat[b] @ Wv_i @ cw) + cb
    attn_map  = ones([B,1,1])

Sharding: one basis i per NeuronCore (NB == 8 == n_cores). Each core streams
its own Wv_i (converted to fp16 on host; 8.4 MB), computes
    A_i = (feat @ Wv_i) @ cw          (device: fp16 matmul, fp32 accum,
                                       PE transpose + fp32 classifier)
plus the hypernetwork -> coeffs_div   (device, fp32, replicated)
Host combine (the unshard step):
    coeffs = softmax(static_coeffs) + coeffs_div
    logits = sum_i coeffs[:, i:i+1] * A_i + cb
"""

import os
from contextlib import ExitStack

import numpy as np

import concourse.bass as bass
import concourse.tile as tile
from concourse import bacc, mybir
from concourse.bass_utils import run_bass_kernel_spmd
from concourse.masks import make_identity

B, D, NB, Z, POOL, C = 16, 2048, 8, 64, 2048, 10
H = 256
P = 128
NCORES = 8
f32 = mybir.dt.float32
f16 = mybir.dt.float16

# Weight dtype for the big matmul: "f16" (half traffic, ~1e-3 rel err) or
# "f32" (exact, PE runs at 1/4 rate and DMA doubles).
W_DTYPE = os.environ.get("KERNEL_W_DTYPE", "f16")

N_TILE = 512          # moving free dim per matmul / output column block
N_BLOCKS = D // N_TILE          # 4
K_TILES = D // P                # 16
JQ = 4                          # j-tiles (of 128 rows) per W DMA chunk
W_BUFS = 8


def _build(w_dt):
    nc = bacc.Bacc(
        "TRN2", target_bir_lowering=False, debug=False, num_devices=NCORES
    )

    wv = nc.dram_tensor("wv", [D, D], w_dt, kind="ExternalInput").ap()
    featT = nc.dram_tensor("featT", [D, B], w_dt, kind="ExternalInput").ap()
    zT = nc.dram_tensor("zT", [Z, B], f32, kind="ExternalInput").ap()
    hw1 = nc.dram_tensor("hw1", [Z, H], f32, kind="ExternalInput").ap()
    hb1 = nc.dram_tensor("hb1", [H], f32, kind="ExternalInput").ap()
    hw2 = nc.dram_tensor("hw2", [H, NB], f32, kind="ExternalInput").ap()
    hb2 = nc.dram_tensor("hb2", [1, NB], f32, kind="ExternalInput").ap()
    cw = nc.dram_tensor("cw", [D, C], f32, kind="ExternalInput").ap()
    out = nc.dram_tensor("out", [B, C], f32, kind="ExternalOutput").ap()
    out_div = nc.dram_tensor("out_div", [B, NB], f32, kind="ExternalOutput").ap()

    with tile.TileContext(nc) as tc, ExitStack() as ctx:
        const = ctx.enter_context(tc.tile_pool(name="const", bufs=1))
        wpool = ctx.enter_context(tc.tile_pool(name="wpool", bufs=W_BUFS))
        sb = ctx.enter_context(tc.tile_pool(name="sb", bufs=2))
        ps_y = ctx.enter_context(tc.tile_pool(name="ps_y", bufs=2, space="PSUM"))
        ps_t = ctx.enter_context(tc.tile_pool(name="ps_t", bufs=2, space="PSUM"))
        ps_o = ctx.enter_context(tc.tile_pool(name="ps_o", bufs=1, space="PSUM"))
        ps_h = ctx.enter_context(tc.tile_pool(name="ps_h", bufs=1, space="PSUM"))

        # ---- small-parameter loads (scalar-engine HWDGE ring; W uses sync) --
        feat_sb = const.tile([P, K_TILES, B], w_dt)
        nc.scalar.dma_start(feat_sb[:], featT.rearrange("(t p) b -> p t b", p=P))
        cw_sb = const.tile([P, K_TILES, C], f32)
        nc.scalar.dma_start(cw_sb[:], cw.rearrange("(t p) c -> p t c", p=P))
        zT_sb = const.tile([Z, B], f32)
        nc.scalar.dma_start(zT_sb[:], zT)
        hw1_sb = const.tile([Z, H], f32)
        nc.scalar.dma_start(hw1_sb[:], hw1)
        hb1_sb = const.tile([P, H // P], f32)
        nc.scalar.dma_start(hb1_sb[:], hb1.rearrange("(t p) -> p t", p=P))
        hw2_sb = const.tile([P, H // P, NB], f32)
        nc.scalar.dma_start(hw2_sb[:], hw2.rearrange("(t p) n -> p t n", p=P))
        hb2_sb = const.tile([1, NB], f32)
        nc.scalar.dma_start(hb2_sb[:], hb2)

        ident = const.tile([B, B], f32)
        make_identity(nc, ident[:])
        ones_row = const.tile([1, B], f32)
        nc.gpsimd.memset(ones_row[:], 1.0)

        # ---- hypernetwork: coeffs_div = softmax(relu(z@hw1+hb1) @ hw2 + hb2)
        hrT = []  # h^T chunks, [128, B], relu'd, in SBUF
        for t in range(H // P):
            ph = ps_h.tile([P, B], f32, name=f"ph{t}")
            nc.tensor.matmul(
                ph[:], lhsT=hw1_sb[:, t * P:(t + 1) * P], rhs=zT_sb[:],
                start=True, stop=True,
            )
            hr = sb.tile([P, B], f32, name=f"hr{t}")
            nc.scalar.activation(
                hr[:], ph[:], mybir.ActivationFunctionType.Relu,
                bias=hb1_sb[:, t:t + 1],
            )
            hrT.append(hr)

        pl = ps_h.tile([B, NB], f32, name="pl")
        nc.tensor.matmul(pl[:], lhsT=hrT[0][:], rhs=hw2_sb[:, 0, :],
                         start=True, stop=False)
        nc.tensor.matmul(pl[:], lhsT=hrT[1][:], rhs=hw2_sb[:, 1, :],
                         start=False, stop=False)
        # rank-1 trick: broadcast-add hb2 over the batch rows
        nc.tensor.matmul(pl[:], lhsT=ones_row[:], rhs=hb2_sb[:],
                         start=False, stop=True)

        m = sb.tile([B, 1], f32, name="m")
        nc.vector.reduce_max(m[:], pl[:], axis=mybir.AxisListType.X)
        negm = sb.tile([B, 1], f32, name="negm")
        nc.vector.tensor_scalar_mul(negm[:], m[:], -1.0)
        e = sb.tile([B, NB], f32, name="e")
        s = sb.tile([B, 1], f32, name="s")
        nc.scalar.activation(e[:], pl[:], mybir.ActivationFunctionType.Exp,
                             bias=negm[:], accum_out=s[:])
        r = sb.tile([B, 1], f32, name="r")
        nc.vector.reciprocal(r[:], s[:])
        div_sb = sb.tile([B, NB], f32, name="div_sb")
        nc.vector.tensor_scalar_mul(div_sb[:], e[:], r[:])
        nc.scalar.dma_start(out_div, div_sb[:])

        # ---- main: Y = feat @ Wv (fp16/fp32), A = Y @ cw (fp32) ------------
        po = ps_o.tile([B, C], f32)
        for nt in range(N_BLOCKS):
            ncol = slice(nt * N_TILE, (nt + 1) * N_TILE)
            py = ps_y.tile([B, N_TILE], f32, name="py")
            for jq in range(K_TILES // JQ):
                wt = wpool.tile([P, JQ, N_TILE], w_dt, name="wt")
                nc.sync.dma_start(
                    wt[:],
                    wv[jq * JQ * P:(jq + 1) * JQ * P, ncol].rearrange(
                        "(jj p) n -> p jj n", p=P
                    ),
                )
                for jj in range(JQ):
                    jt = jq * JQ + jj
                    nc.tensor.matmul(
                        py[:], lhsT=feat_sb[:, jt, :], rhs=wt[:, jj, :],
                        start=(jt == 0), stop=(jt == K_TILES - 1),
                    )
            ysb = sb.tile([B, N_TILE], f32, name="ysb")
            nc.scalar.copy(ysb[:], py[:])
            for ss in range(N_TILE // P):
                kt = nt * (N_TILE // P) + ss
                pt = ps_t.tile([P, B], f32, name="pt")
                nc.tensor.transpose(
                    pt[:], ysb[:, ss * P:(ss + 1) * P], ident[:]
                )
                yt = sb.tile([P, B], f32, name="yt")
                nc.scalar.copy(yt[:], pt[:])
                nc.tensor.matmul(
                    po[:], lhsT=yt[:], rhs=cw_sb[:, kt, :],
                    start=(kt == 0), stop=(kt == K_TILES - 1),
                )

        osb = sb.tile([B, C], f32, name="osb")
        nc.scalar.copy(osb[:], po[:])
        nc.scalar.dma_start(out, osb[:])

    nc.compile()
    return nc


_CACHE = {}


def _get_program():
    if W_DTYPE not in _CACHE:
        _CACHE[W_DTYPE] = _build(f16 if W_DTYPE == "f16" else f32)
    return _CACHE[W_DTYPE]


def _np_softmax(x, axis):
    x = x - x.max(axis=axis, keepdims=True)
    e = np.exp(x)
    return e / e.sum(axis=axis, keepdims=True)


def kernel(features, seeds, seed_emb, static_coeffs, hw1, hb1, hw2, hb2,
           wq, wk, wv, cw, cb, _run_kwargs=None, _results_out=None):
    features = np.asarray(features, dtype=np.float32)
    seeds = np.asarray(seeds).astype(np.int64)
    seed_emb = np.asarray(seed_emb, dtype=np.float32)
    static_coeffs = np.asarray(static_coeffs, dtype=np.float32)
    hw1 = np.ascontiguousarray(np.asarray(hw1, dtype=np.float32))
    hb1 = np.ascontiguousarray(np.asarray(hb1, dtype=np.float32))
    hw2 = np.ascontiguousarray(np.asarray(hw2, dtype=np.float32))
    hb2 = np.ascontiguousarray(np.asarray(hb2, dtype=np.float32)).reshape(1, NB)
    wv = np.asarray(wv, dtype=np.float32)
    cw = np.ascontiguousarray(np.asarray(cw, dtype=np.float32))
    cb = np.asarray(cb, dtype=np.float32)

    np_wdt = np.float16 if W_DTYPE == "f16" else np.float32
    featT = np.ascontiguousarray(features.T.astype(np_wdt))
    zT = np.ascontiguousarray(seed_emb[seeds].T)  # [Z, B]

    shared = {
        "featT": featT, "zT": zT, "hw1": hw1, "hb1": hb1,
        "hw2": hw2, "hb2": hb2, "cw": cw,
    }
    in_maps = [
        {**shared, "wv": np.ascontiguousarray(wv[i].astype(np_wdt))}
        for i in range(NCORES)
    ]

    nc = _get_program()
    res = run_bass_kernel_spmd(
        nc, in_maps, core_ids=list(range(NCORES)), **(_run_kwargs or {})
    )
    if _results_out is not None:
        _results_out.append(res)

    coeffs = _np_softmax(static_coeffs, 1) + res.results[0]["out_div"]  # [B, NB]
    logits = np.zeros((B, C), np.float32)
    for i in range(NCORES):
        logits += coeffs[:, i:i + 1] * res.results[i]["out"]
    logits += cb
    attn_map = np.ones((B, 1, 1), np.float32)
    return logits, attn_map


# revision 3
# speedup vs baseline: 1.3108x; 1.3108x over previous
"""Trainium2 Bass kernel for nn_DynamicAttentionModel.

Model math (see reference):
    z          = seed_emb[seeds]                          [B, Z]
    h          = relu(z @ hw1 + hb1)                      [B, H]
    coeffs_div = softmax(h @ hw2 + hb2, axis=1)           [B, NB]
    coeffs     = softmax(static_coeffs, 1) + coeffs_div   [B, NB]
    q/k/v      = einsum('bi,bj,ijk->bk', coeffs, feat, W{q,k,v})
    scores     = <q,k>/sqrt(D); attn = softmax over a SINGLE element == 1.0
    pooled     = attn * v == v
    logits     = pooled @ cw + cb

Because attn_map is softmax over one element it is identically 1.0, so q and k
never influence the output: logits depends only on the v-projection.

    logits[b] = sum_i coeffs[b,i] * (feat[b] @ Wv_i @ cw) + cb
    attn_map  = ones([B,1,1])

Sharding: one basis i per NeuronCore (NB == 8 == n_cores). Each core streams
its own Wv_i (fp16 on host; 8.4 MB — the memory-roofline term), computes
    A_i = (feat @ Wv_i) @ cw        fp16 matmul, fp32 accum, PE-transpose +
                                    fp32 classifier, all on device
    e   = exp(relu(z@hw1+hb1) @ hw2 + hb2)     (device, fp32, replicated;
                                    range of the logits is ~[-3,3] so the
                                    max-subtraction is unnecessary)
Host combine (the unshard step):
    coeffs = softmax(static_coeffs) + e / e.sum(1)
    logits = sum_i coeffs[:, i:i+1] * A_i + cb

Only three engines are used (PE, ACT, SP) to minimize Tile's start/end
barrier cost; all DRAM operands are host-pre-tiled so every DMA has >=4KB
contiguous runs per partition.
"""

import os
from contextlib import ExitStack

import numpy as np

import concourse.bass as bass
import concourse.tile as tile
from concourse import bacc, mybir
from concourse.bass_utils import run_bass_kernel_spmd

B, D, NB, Z, POOL, C = 16, 2048, 8, 64, 2048, 10
H = 256
P = 128
NCORES = 8
f32 = mybir.dt.float32
f16 = mybir.dt.float16

W_DTYPE = os.environ.get("KERNEL_W_DTYPE", "f16")

N_TILE = 512                    # output column block (one PSUM bank)
N_BLOCKS = D // N_TILE          # 4
K_TILES = D // P                # 16
JQ = 4                          # j-tiles of 128 rows per W DMA chunk
W_BUFS = 10


def _build(w_dt):
    nc = bacc.Bacc(
        "TRN2", target_bir_lowering=False, debug=False, num_devices=NCORES
    )

    # wv pre-tiled on host: [nt, jq, p, jj*N_TILE] so each DMA chunk is
    # [128, JQ*N_TILE] with 4KB (fp16) contiguous per partition.
    wv = nc.dram_tensor(
        "wv", [N_BLOCKS, K_TILES // JQ, P, JQ * N_TILE], w_dt,
        kind="ExternalInput",
    ).ap()
    featT = nc.dram_tensor("featT", [P, K_TILES * B], w_dt, kind="ExternalInput").ap()
    cwT = nc.dram_tensor("cwT", [P, K_TILES * C], f32, kind="ExternalInput").ap()
    zT = nc.dram_tensor("zT", [Z, B], f32, kind="ExternalInput").ap()
    hw1 = nc.dram_tensor("hw1", [Z, H], f32, kind="ExternalInput").ap()
    hb1 = nc.dram_tensor("hb1", [P, H // P], f32, kind="ExternalInput").ap()
    hw2 = nc.dram_tensor("hw2", [P, (H // P) * NB], f32, kind="ExternalInput").ap()
    hb2 = nc.dram_tensor("hb2", [1, NB], f32, kind="ExternalInput").ap()
    ident = nc.dram_tensor("ident", [B, B], f32, kind="ExternalInput").ap()
    ones_row = nc.dram_tensor("ones_row", [1, B], f32, kind="ExternalInput").ap()
    out = nc.dram_tensor("out", [B, C], f32, kind="ExternalOutput").ap()
    out_e = nc.dram_tensor("out_e", [B, NB], f32, kind="ExternalOutput").ap()

    with tile.TileContext(nc) as tc, ExitStack() as ctx:
        const = ctx.enter_context(tc.tile_pool(name="const", bufs=1))
        wpool = ctx.enter_context(tc.tile_pool(name="wpool", bufs=W_BUFS))
        sb = ctx.enter_context(tc.tile_pool(name="sb", bufs=2))
        ps_y = ctx.enter_context(tc.tile_pool(name="ps_y", bufs=2, space="PSUM"))
        ps_t = ctx.enter_context(tc.tile_pool(name="ps_t", bufs=2, space="PSUM"))
        ps_o = ctx.enter_context(tc.tile_pool(name="ps_o", bufs=1, space="PSUM"))
        ps_h = ctx.enter_context(tc.tile_pool(name="ps_h", bufs=1, space="PSUM"))

        # ---- small parameters on the scalar (ACT) HWDGE ring, up front ----
        feat_sb = const.tile([P, K_TILES, B], w_dt)
        nc.scalar.dma_start(feat_sb[:], featT.rearrange("p (t b) -> p t b", t=K_TILES))
        cw_sb = const.tile([P, K_TILES, C], f32)
        nc.scalar.dma_start(cw_sb[:], cwT.rearrange("p (t c) -> p t c", t=K_TILES))
        zT_sb = const.tile([Z, B], f32)
        nc.scalar.dma_start(zT_sb[:], zT)
        hw1_sb = const.tile([Z, H], f32)
        nc.scalar.dma_start(hw1_sb[:], hw1)
        hb1_sb = const.tile([P, H // P], f32)
        nc.scalar.dma_start(hb1_sb[:], hb1)
        hw2_sb = const.tile([P, H // P, NB], f32)
        nc.scalar.dma_start(hw2_sb[:], hw2.rearrange("p (t n) -> p t n", t=H // P))
        hb2_sb = const.tile([1, NB], f32)
        nc.scalar.dma_start(hb2_sb[:], hb2)
        ident_sb = const.tile([B, B], f32)
        nc.scalar.dma_start(ident_sb[:], ident)
        ones_sb = const.tile([1, B], f32)
        nc.scalar.dma_start(ones_sb[:], ones_row)

        # ---- hypernetwork: e = exp(relu(z@hw1+hb1) @ hw2 + hb2) -----------
        hrT = []
        for t in range(H // P):
            ph = ps_h.tile([P, B], f32, name=f"ph{t}")
            nc.tensor.matmul(
                ph[:], lhsT=hw1_sb[:, t * P:(t + 1) * P], rhs=zT_sb[:],
                start=True, stop=True,
            )
            hr = sb.tile([P, B], f32, name=f"hr{t}")
            nc.scalar.activation(
                hr[:], ph[:], mybir.ActivationFunctionType.Relu,
                bias=hb1_sb[:, t:t + 1],
            )
            hrT.append(hr)

        pl = ps_h.tile([B, NB], f32, name="pl")
        nc.tensor.matmul(pl[:], lhsT=hrT[0][:], rhs=hw2_sb[:, 0, :],
                         start=True, stop=False)
        nc.tensor.matmul(pl[:], lhsT=hrT[1][:], rhs=hw2_sb[:, 1, :],
                         start=False, stop=False)
        # rank-1 trick: broadcast-add hb2 over the batch rows
        nc.tensor.matmul(pl[:], lhsT=ones_sb[:], rhs=hb2_sb[:],
                         start=False, stop=True)
        e_sb = sb.tile([B, NB], f32, name="e_sb")
        nc.scalar.activation(e_sb[:], pl[:], mybir.ActivationFunctionType.Exp)
        nc.scalar.dma_start(out_e, e_sb[:])

        # ---- main: Y = feat @ Wv (fp16), A = Y @ cw (fp32) ----------------
        po = ps_o.tile([B, C], f32)
        for nt in range(N_BLOCKS):
            py = ps_y.tile([B, N_TILE], f32, name="py")
            for jq in range(K_TILES // JQ):
                wt = wpool.tile([P, JQ * N_TILE], w_dt, name="wt")
                nc.sync.dma_start(wt[:], wv[nt, jq])
                for jj in range(JQ):
                    jt = jq * JQ + jj
                    nc.tensor.matmul(
                        py[:], lhsT=feat_sb[:, jt, :],
                        rhs=wt[:, jj * N_TILE:(jj + 1) * N_TILE],
                        start=(jt == 0), stop=(jt == K_TILES - 1),
                    )
            ysb = sb.tile([B, N_TILE], f32, name="ysb")
            nc.scalar.copy(ysb[:], py[:])
            for ss in range(N_TILE // P):
                kt = nt * (N_TILE // P) + ss
                pt = ps_t.tile([P, B], f32, name="pt")
                nc.tensor.transpose(
                    pt[:], ysb[:, ss * P:(ss + 1) * P], ident_sb[:]
                )
                yt = sb.tile([P, B], f32, name="yt")
                nc.scalar.copy(yt[:], pt[:])
                nc.tensor.matmul(
                    po[:], lhsT=yt[:], rhs=cw_sb[:, kt, :],
                    start=(kt == 0), stop=(kt == K_TILES - 1),
                )

        osb = sb.tile([B, C], f32, name="osb")
        nc.scalar.copy(osb[:], po[:])
        nc.sync.dma_start(out, osb[:])

    nc.compile()
    return nc


_CACHE = {}


def _get_program():
    if W_DTYPE not in _CACHE:
        _CACHE[W_DTYPE] = _build(f16 if W_DTYPE == "f16" else f32)
    return _CACHE[W_DTYPE]


def _np_softmax(x, axis):
    x = x - x.max(axis=axis, keepdims=True)
    e = np.exp(x)
    return e / e.sum(axis=axis, keepdims=True)


def _tile_w(w, np_wdt):
    # [D, D] -> [nt, jq, p, jj*N]  with  [p, jj*N+n] = w[jq*JQ*P + jj*P + p,
    #                                                    nt*N_TILE + n]
    t = w.reshape(K_TILES // JQ, JQ, P, N_BLOCKS, N_TILE)
    t = t.transpose(3, 0, 2, 1, 4).reshape(
        N_BLOCKS, K_TILES // JQ, P, JQ * N_TILE
    )
    return np.ascontiguousarray(t.astype(np_wdt))


def _tile_rows(x, np_dt):
    # [D, M] -> [p, t*M] with [p, t*M+m] = x[t*P+p, m]
    n, m = x.shape
    t = x.reshape(n // P, P, m).transpose(1, 0, 2).reshape(P, (n // P) * m)
    return np.ascontiguousarray(t.astype(np_dt))


def kernel(features, seeds, seed_emb, static_coeffs, hw1, hb1, hw2, hb2,
           wq, wk, wv, cw, cb, _run_kwargs=None, _results_out=None):
    features = np.asarray(features, dtype=np.float32)
    seeds = np.asarray(seeds).astype(np.int64)
    seed_emb = np.asarray(seed_emb, dtype=np.float32)
    static_coeffs = np.asarray(static_coeffs, dtype=np.float32)
    hw1 = np.ascontiguousarray(np.asarray(hw1, dtype=np.float32))
    hb1 = np.asarray(hb1, dtype=np.float32)
    hw2 = np.asarray(hw2, dtype=np.float32)
    hb2 = np.ascontiguousarray(np.asarray(hb2, dtype=np.float32)).reshape(1, NB)
    wv = np.asarray(wv, dtype=np.float32)
    cw = np.asarray(cw, dtype=np.float32)
    cb = np.asarray(cb, dtype=np.float32)

    np_wdt = np.float16 if W_DTYPE == "f16" else np.float32
    shared = {
        "featT": _tile_rows(features.T, np_wdt),
        "cwT": _tile_rows(cw, np.float32),
        "zT": np.ascontiguousarray(seed_emb[seeds].T),
        "hw1": hw1,
        "hb1": np.ascontiguousarray(
            hb1.reshape(H // P, P).T.astype(np.float32)
        ),
        "hw2": _tile_rows(hw2, np.float32),
        "hb2": hb2,
        "ident": np.eye(B, dtype=np.float32),
        "ones_row": np.ones((1, B), np.float32),
    }
    in_maps = [{**shared, "wv": _tile_w(wv[i], np_wdt)} for i in range(NCORES)]

    nc = _get_program()
    res = run_bass_kernel_spmd(
        nc, in_maps, core_ids=list(range(NCORES)), **(_run_kwargs or {})
    )
    if _results_out is not None:
        _results_out.append(res)

    e = res.results[0]["out_e"]
    coeffs = _np_softmax(static_coeffs, 1) + e / e.sum(axis=1, keepdims=True)
    logits = np.zeros((B, C), np.float32)
    for i in range(NCORES):
        logits += coeffs[:, i:i + 1] * res.results[i]["out"]
    logits += cb
    attn_map = np.ones((B, 1, 1), np.float32)
    return logits, attn_map
